# revision 2
# baseline (speedup 1.0000x reference)
"""Trainium2 Bass kernel for a causal single-head attention layer.

reference:
    v = inp @ Wv + bv; k = inp @ Wk + bk; q = inp @ Wq + bq      # [B,T,H]
    W = softmax(causal_mask(k @ q^T / sqrt(C)))                  # [B,T,T]
    out = W @ v                                                  # [B,T,H]

B=512, T=256, C=384, H=64. Pure data parallel over 8 NeuronCores
(64 batches each). Inside a core, batches are processed in pairs so the
projection matmuls run with a 512-wide moving operand.

Layout trick: scores are computed transposed (S^T[s,t], s on partitions)
with lhsT=q^T slices / rhs=k^T. exp(S^T) in that layout is directly the
stationary operand for the P@V matmul, so no on-chip transposes anywhere
(the host pre-transposes inp to [B, C, T]). Softmax normalization is done
by appending a ones-column to V, which makes the P@V matmul also emit the
row sums; a per-partition reciprocal multiply finishes the softmax.
"""

import math
import re

import numpy as np
import ml_dtypes

import concourse.bass as bass
import concourse.bacc as bacc
import concourse.mybir as mybir
import concourse.tile as tile
from concourse.bass_utils import run_bass_kernel_spmd

# ---------------------------------------------------------------------------
# Workaround: this toolchain's TileContext tail drain can carry >1 sync wait
# on a single Drain instruction, which walrus codegen rejects ("Too many sync
# wait commands"). Split the waits across SP NOPs (HW allows 1 wait/inst).
from bass_rust import ScopedClock, VectorClock


def _patched_drain_and_barrier(self, tick_clock, wait_clock):
    gc = tick_clock.global_clock
    ticks = eval(re.match(r"VectorClock\((\[.*\])\)", repr(gc)).group(1))
    for proc, t in enumerate(ticks):
        if t > 0:
            v = [0] * len(ticks)
            v[proc] = t
            nop = self.nc.sync.nop(nofuse=True, hint="tail_drain_wait")
            wait_clock.add_sem_waits(nop.ins, ScopedClock({None: VectorClock(v)}))
    self.nc.sync.drain()
    self.nc.all_engine_barrier()
    popped = self.nc._tile_sem_poison_stack.pop()
    assert popped is self._sem_poison
    self.nc.clear_and_free_semaphores(list(self.sems.allocated().values()))
    self.nc.all_engine_barrier()


tile.TileContext._drain_and_barrier = _patched_drain_and_barrier
# ---------------------------------------------------------------------------

N_CORES = 8
B, T, C, H = 512, 256, 384, 64
NB = B // N_CORES          # batches per core
KC = C // 128              # contraction chunks
SCALE = C ** (-0.5)
F32 = mybir.dt.float32
BF16 = mybir.dt.bfloat16
AF = mybir.ActivationFunctionType


def build_nc():
    nc = bacc.Bacc("TRN2", target_bir_lowering=False, debug=False)
    x_h = nc.declare_dram_parameter("x", [NB, C, T], BF16, isOutput=False)
    wq_h = nc.declare_dram_parameter("wq", [C, H], BF16, isOutput=False)
    wk_h = nc.declare_dram_parameter("wk", [C, H], BF16, isOutput=False)
    wv_h = nc.declare_dram_parameter("wv", [C, H], BF16, isOutput=False)
    bq_h = nc.declare_dram_parameter("bq", [H, 1], F32, isOutput=False)
    bk_h = nc.declare_dram_parameter("bk", [H, 1], F32, isOutput=False)
    bvb_h = nc.declare_dram_parameter("bvb", [128, H], F32, isOutput=False)
    out_h = nc.declare_dram_parameter("out", [NB, T, H], F32, isOutput=True)

    with tile.TileContext(nc) as tc:
        with (
            tc.tile_pool(name="const", bufs=1) as const,
            tc.tile_pool(name="xp", bufs=3) as xp,
            tc.tile_pool(name="qkp", bufs=3) as qkp,
            tc.tile_pool(name="exp", bufs=4) as expp,
            tc.tile_pool(name="vp", bufs=3) as vp,
            tc.tile_pool(name="op", bufs=4) as op,
            tc.tile_pool(name="ps_qk", bufs=2, space="PSUM") as ps_qk,
            tc.tile_pool(name="ps_v", bufs=2, space="PSUM") as ps_v,
            tc.tile_pool(name="ps_att", bufs=2, space="PSUM") as ps_att,
        ):
            wq_sb = const.tile([128, KC, H], BF16, tag="wq")
            nc.sync.dma_start(wq_sb[:], wq_h.ap().rearrange("(k p) h -> p k h", p=128))
            wk_sb = const.tile([128, KC, H], BF16, tag="wk")
            nc.sync.dma_start(wk_sb[:], wk_h.ap().rearrange("(k p) h -> p k h", p=128))
            wv_sb = const.tile([128, KC, H], BF16, tag="wv")
            nc.sync.dma_start(wv_sb[:], wv_h.ap().rearrange("(k p) h -> p k h", p=128))
            bq_sb = const.tile([H, 1], F32, tag="bq")
            nc.sync.dma_start(bq_sb[:], bq_h.ap())
            bk_sb = const.tile([H, 1], F32, tag="bk")
            nc.sync.dma_start(bk_sb[:], bk_h.ap())
            bvb_sb = const.tile([128, H], F32, tag="bvb")
            nc.sync.dma_start(bvb_sb[:], bvb_h.ap())

            for g in range(NB // 2):
                # ---- load x^T for both batches: [c_part, k, j, t] -------
                xt = xp.tile([128, KC, 2, T], BF16, tag="xt", name=f"xt{g}")
                for j in range(2):
                    nc.sync.dma_start(
                        xt[:, :, j, :],
                        x_h.ap()[2 * g + j].rearrange("(k p) t -> p k t", p=128),
                    )

                # ---- q^T, k^T projections (both batches at once) --------
                q_ps = ps_qk.tile([H, 2, T], F32, tag="qk", name=f"qps{g}")
                k_ps = ps_qk.tile([H, 2, T], F32, tag="qk", name=f"kps{g}")
                for k in range(KC):
                    nc.tensor.matmul(
                        q_ps[:], wq_sb[:, k, :], xt[:, k],
                        start=(k == 0), stop=(k == KC - 1),
                    )
                for k in range(KC):
                    nc.tensor.matmul(
                        k_ps[:], wk_sb[:, k, :], xt[:, k],
                        start=(k == 0), stop=(k == KC - 1),
                    )
                qt = qkp.tile([H, 2, T], BF16, tag="qt", name=f"qt{g}")
                nc.scalar.activation(qt[:], q_ps[:], AF.Identity, bias=bq_sb[:])
                kt = qkp.tile([H, 2, T], BF16, tag="kt", name=f"kt{g}")
                nc.vector.tensor_scalar_add(kt[:], k_ps[:], bk_sb[:])

                # ---- v in [s, h] layout (x^T chunks stationary) ---------
                v_ps = ps_v.tile([128, 2, 2, H], F32, tag="v", name=f"vps{g}")
                for j in range(2):
                    for si in range(2):
                        for k in range(KC):
                            nc.tensor.matmul(
                                v_ps[:, j, si, :],
                                xt[:, k, j, si * 128:(si + 1) * 128],
                                wv_sb[:, k, :],
                                start=(k == 0), stop=(k == KC - 1),
                            )
                vo = vp.tile([128, 2, 2, H + 1], BF16, tag="vo", name=f"vo{g}")
                nc.gpsimd.memset(vo[:, :, :, H:H + 1], 1.0)
                for j in range(2):
                    for si in range(2):
                        nc.vector.tensor_add(
                            vo[:, j, si, 0:H], v_ps[:, j, si, :], bvb_sb[:]
                        )

                for j in range(2):
                    # ---- scores^T: cols 0:256 = s-chunk0 (t 0:256),
                    #                cols 256:384 = s-chunk1 (t 128:256) ---
                    st_ps = ps_att.tile([128, 384], F32, tag="att", name=f"st{g}_{j}")
                    nc.tensor.matmul(
                        st_ps[:, 0:T], qt[:, j, 0:128], kt[:, j, :],
                        start=True, stop=True,
                    )
                    nc.tensor.matmul(
                        st_ps[:, T:T + 128], qt[:, j, 128:T], kt[:, j, 128:T],
                        start=True, stop=True,
                    )
                    # ---- exp + causal mask ------------------------------
                    ex = expp.tile([128, 384], BF16, tag="ex", name=f"ex{g}_{j}")
                    nc.scalar.activation(ex[:], st_ps[:], AF.Exp, scale=SCALE)
                    nc.gpsimd.affine_select(
                        out=ex[:, 0:T], in_=ex[:, 0:T],
                        compare_op=mybir.AluOpType.is_ge, fill=0.0,
                        base=0, pattern=[[1, T]], channel_multiplier=-1,
                    )
                    nc.gpsimd.affine_select(
                        out=ex[:, T:T + 128], in_=ex[:, T:T + 128],
                        compare_op=mybir.AluOpType.is_ge, fill=0.0,
                        base=0, pattern=[[1, 128]], channel_multiplier=-1,
                    )
                    # ---- out = P @ [v | 1] ------------------------------
                    ou_ps = ps_att.tile([128, 2, H + 1], F32, tag="att", name=f"ou{g}_{j}")
                    nc.tensor.matmul(
                        ou_ps[:, 0, :], ex[:, 0:128], vo[:, j, 0, :],
                        start=True, stop=True,
                    )
                    nc.tensor.matmul(
                        ou_ps[:, 1, :], ex[:, 128:T], vo[:, j, 0, :],
                        start=True, stop=False,
                    )
                    nc.tensor.matmul(
                        ou_ps[:, 1, :], ex[:, T:T + 128], vo[:, j, 1, :],
                        start=False, stop=True,
                    )
                    # ---- normalize + store ------------------------------
                    rec = op.tile([128, 2, 1], F32, tag="rec", name=f"rec{g}_{j}")
                    ot = op.tile([128, 2, H], F32, tag="ot", name=f"ot{g}_{j}")
                    for u in range(2):
                        nc.vector.reciprocal(rec[:, u, :], ou_ps[:, u, H:H + 1])
                        nc.scalar.activation(
                            ot[:, u, :], ou_ps[:, u, 0:H], AF.Copy,
                            scale=rec[:, u, :],
                        )
                    nc.sync.dma_start(
                        out_h.ap()[2 * g + j].rearrange("(u p) h -> p u h", p=128),
                        ot[:],
                    )
    nc.compile()
    return nc


_NC_CACHE = None


def _get_nc():
    global _NC_CACHE
    if _NC_CACHE is None:
        _NC_CACHE = build_nc()
    return _NC_CACHE


def kernel(inp, Wv, bv, Wk, bk, Wq, bq):
    inp = np.asarray(inp, dtype=np.float32)
    Wv = np.asarray(Wv, dtype=np.float32)
    Wk = np.asarray(Wk, dtype=np.float32)
    Wq = np.asarray(Wq, dtype=np.float32)
    bv = np.asarray(bv, dtype=np.float32)
    bk = np.asarray(bk, dtype=np.float32)
    bq = np.asarray(bq, dtype=np.float32)

    bf16 = ml_dtypes.bfloat16
    wq_b = np.ascontiguousarray(Wq.astype(bf16))
    wk_b = np.ascontiguousarray(Wk.astype(bf16))
    wv_b = np.ascontiguousarray(Wv.astype(bf16))
    bq_c = np.ascontiguousarray(bq.reshape(H, 1))
    bk_c = np.ascontiguousarray(bk.reshape(H, 1))
    bvb = np.ascontiguousarray(np.tile(bv.reshape(1, H), (128, 1)))

    in_maps = []
    for c in range(N_CORES):
        shard = inp[c * NB:(c + 1) * NB]                  # [NB, T, C]
        x_t = np.ascontiguousarray(
            shard.transpose(0, 2, 1).astype(bf16)          # [NB, C, T]
        )
        in_maps.append({
            "x": x_t, "wq": wq_b, "wk": wk_b, "wv": wv_b,
            "bq": bq_c, "bk": bk_c, "bvb": bvb,
        })

    nc = _get_nc()
    res = run_bass_kernel_spmd(nc, in_maps, core_ids=list(range(N_CORES)))
    return np.concatenate(
        [res.results[c]["out"] for c in range(N_CORES)], axis=0
    )


# revision 4
# speedup vs baseline: 1.7915x; 1.7915x over previous
"""Trainium2 Bass kernel for a causal single-head attention layer.

reference:
    v = inp @ Wv + bv; k = inp @ Wk + bk; q = inp @ Wq + bq      # [B,T,H]
    W = softmax(causal_mask(k @ q^T / sqrt(C)))                  # [B,T,T]
    out = W @ v                                                  # [B,T,H]

B=512, T=256, C=384, H=64. Pure data parallel over 8 NeuronCores
(64 batches each). Inside a core, batches are processed in pairs so the
projection matmuls run with a 512-wide moving operand.

Layout trick: scores are computed transposed (S^T[s,t], s on partitions)
with lhsT=q^T slices / rhs=k^T. exp(S^T) in that layout is directly the
stationary operand for the P@V matmul, so no on-chip transposes anywhere
(the host pre-transposes inp to [B, C, T]). Softmax normalization is done
by appending a ones-column to V, which makes the P@V matmul also emit the
row sums; a per-partition reciprocal multiply finishes the softmax.
The causal mask is a precomputed 0/1 bf16 tile multiplied in after exp
(max-subtraction is skipped: |scores/sqrt(C)| < ~3 for this problem size,
so exp cannot overflow and softmax is shift-invariant anyway).
"""

import numpy as np
import ml_dtypes

import concourse.bass as bass
import concourse.bacc as bacc
import concourse.mybir as mybir
import concourse.tile as tile
from concourse.bass import broadcast_tensor_aps
from concourse.bass_utils import run_bass_kernel_spmd

N_CORES = 8
B, T, C, H = 512, 256, 384, 64
NB = B // N_CORES          # batches per core
KC = C // 128              # contraction chunks
SCALE = C ** (-0.5)
F32 = mybir.dt.float32
BF16 = mybir.dt.bfloat16
AF = mybir.ActivationFunctionType


def _bmul(nc, out, a, b):
    """tensor_tensor multiply with numpy-style broadcast of b onto a."""
    a2, b2 = broadcast_tensor_aps(a, b)
    nc.vector.tensor_tensor(out, a2, b2, op=mybir.AluOpType.mult)


def _badd(nc, out, a, b):
    a2, b2 = broadcast_tensor_aps(a, b)
    nc.vector.tensor_tensor(out, a2, b2, op=mybir.AluOpType.add)


def build_nc():
    nc = bacc.Bacc("TRN2", target_bir_lowering=False, debug=False)
    x_h = nc.declare_dram_parameter("x", [NB, C, T], BF16, isOutput=False)
    wq_h = nc.declare_dram_parameter("wq", [C, H], BF16, isOutput=False)
    wk_h = nc.declare_dram_parameter("wk", [C, H], BF16, isOutput=False)
    wv_h = nc.declare_dram_parameter("wv", [C, H], BF16, isOutput=False)
    bq_h = nc.declare_dram_parameter("bq", [H, 1], F32, isOutput=False)
    bk_h = nc.declare_dram_parameter("bk", [H, 1], F32, isOutput=False)
    bvb_h = nc.declare_dram_parameter("bvb", [128, H], F32, isOutput=False)
    out_h = nc.declare_dram_parameter("out", [NB, T, H], F32, isOutput=True)

    with tile.TileContext(nc) as tc:
        with (
            tc.tile_pool(name="const", bufs=1) as const,
            tc.tile_pool(name="xp", bufs=4) as xp,
            tc.tile_pool(name="qkp", bufs=3) as qkp,
            tc.tile_pool(name="exp", bufs=4) as expp,
            tc.tile_pool(name="vp", bufs=3) as vp,
            tc.tile_pool(name="op", bufs=3) as op,
            tc.tile_pool(name="ps_qk", bufs=4, space="PSUM") as ps_qk,
            tc.tile_pool(name="ps_v", bufs=2, space="PSUM") as ps_v,
            tc.tile_pool(name="ps_att", bufs=2, space="PSUM") as ps_att,
        ):
            wq_sb = const.tile([128, KC, H], BF16, tag="wq")
            nc.sync.dma_start(wq_sb[:], wq_h.ap().rearrange("(k p) h -> p k h", p=128))
            wk_sb = const.tile([128, KC, H], BF16, tag="wk")
            nc.sync.dma_start(wk_sb[:], wk_h.ap().rearrange("(k p) h -> p k h", p=128))
            wv_sb = const.tile([128, KC, H], BF16, tag="wv")
            nc.sync.dma_start(wv_sb[:], wv_h.ap().rearrange("(k p) h -> p k h", p=128))
            bq_sb = const.tile([H, 1], F32, tag="bq")
            nc.sync.dma_start(bq_sb[:], bq_h.ap())
            bk_sb = const.tile([H, 1], F32, tag="bk")
            nc.sync.dma_start(bk_sb[:], bk_h.ap())
            bvb_sb = const.tile([128, H], F32, tag="bvb")
            nc.sync.dma_start(bvb_sb[:], bvb_h.ap())

            # causal mask in the S^T layout: col block 0:256 is s-chunk0 over
            # t in 0:256 (keep when t >= s); block 256:384 is s-chunk1 over
            # t in 128:256 (keep when (t-128) >= (s-128)).
            mask_sb = const.tile([128, 384], BF16, tag="mask")
            nc.gpsimd.memset(mask_sb[:], 1.0)
            nc.gpsimd.affine_select(
                out=mask_sb[:, 0:T], in_=mask_sb[:, 0:T],
                compare_op=mybir.AluOpType.is_ge, fill=0.0,
                base=0, pattern=[[1, T]], channel_multiplier=-1,
            )
            nc.gpsimd.affine_select(
                out=mask_sb[:, T:T + 128], in_=mask_sb[:, T:T + 128],
                compare_op=mybir.AluOpType.is_ge, fill=0.0,
                base=0, pattern=[[1, 128]], channel_multiplier=-1,
            )

            for g in range(NB // 2):
                # ---- load x^T for both batches: [c_part, k, j, t] -------
                xt = xp.tile([128, 2, KC, T], BF16, tag="xt", name=f"xt{g}")
                nc.sync.dma_start(
                    xt[:],
                    x_h.ap()[2 * g:2 * g + 2].rearrange(
                        "j (k p) t -> p j k t", p=128
                    ),
                )

                # ---- q^T, k^T projections (both batches at once) --------
                q_ps = ps_qk.tile([H, 2, T], F32, tag="qk", name=f"qps{g}")
                k_ps = ps_qk.tile([H, 2, T], F32, tag="qk", name=f"kps{g}")
                for k in range(KC):
                    nc.tensor.matmul(
                        q_ps[:], wq_sb[:, k, :], xt[:, :, k, :],
                        start=(k == 0), stop=(k == KC - 1),
                    )
                for k in range(KC):
                    nc.tensor.matmul(
                        k_ps[:], wk_sb[:, k, :], xt[:, :, k, :],
                        start=(k == 0), stop=(k == KC - 1),
                    )
                qt = qkp.tile([H, 2, T], BF16, tag="qt", name=f"qt{g}")
                nc.scalar.activation(qt[:], q_ps[:], AF.Identity, bias=bq_sb[:])
                kt = qkp.tile([H, 2, T], BF16, tag="kt", name=f"kt{g}")
                nc.vector.tensor_scalar_add(kt[:], k_ps[:], bk_sb[:])

                # ---- v in [s, h] layout (x^T chunks stationary) ---------
                v_ps = ps_v.tile([128, 2, 2, H], F32, tag="v", name=f"vps{g}")
                for j in range(2):
                    for si in range(2):
                        for k in range(KC):
                            nc.tensor.matmul(
                                v_ps[:, j, si, :],
                                xt[:, j, k, si * 128:(si + 1) * 128],
                                wv_sb[:, k, :],
                                start=(k == 0), stop=(k == KC - 1),
                            )
                vo = vp.tile([128, 2, 2, H + 1], BF16, tag="vo", name=f"vo{g}")
                nc.gpsimd.memset(vo[:, :, :, H:H + 1], 1.0)
                _badd(nc, vo[:, :, :, 0:H], v_ps[:], bvb_sb[:][:, None, None, :])

                # ---- attention (per batch) ------------------------------
                exs = []
                for j in range(2):
                    st_ps = ps_att.tile([128, 384], F32, tag="att", name=f"st{g}_{j}")
                    nc.tensor.matmul(
                        st_ps[:, 0:T], qt[:, j, 0:128], kt[:, j, :],
                        start=True, stop=True,
                    )
                    nc.tensor.matmul(
                        st_ps[:, T:T + 128], qt[:, j, 128:T], kt[:, j, 128:T],
                        start=True, stop=True,
                    )
                    ex = expp.tile([128, 384], BF16, tag="ex", name=f"ex{g}_{j}")
                    nc.scalar.activation(ex[:], st_ps[:], AF.Exp, scale=SCALE)
                    nc.vector.tensor_mul(ex[:], ex[:], mask_sb[:])
                    exs.append(ex)

                ou_ps = ps_att.tile([128, 2, 2, H + 1], F32, tag="att", name=f"ou{g}")
                for j in range(2):
                    ex = exs[j]
                    nc.tensor.matmul(
                        ou_ps[:, j, 0, :], ex[:, 0:128], vo[:, j, 0, :],
                        start=True, stop=True,
                    )
                    nc.tensor.matmul(
                        ou_ps[:, j, 1, :], ex[:, 128:T], vo[:, j, 0, :],
                        start=True, stop=False,
                    )
                    nc.tensor.matmul(
                        ou_ps[:, j, 1, :], ex[:, T:T + 128], vo[:, j, 1, :],
                        start=False, stop=True,
                    )

                # ---- normalize + store (both batches at once) -----------
                rec = op.tile([128, 2, 2, 1], F32, tag="rec", name=f"rec{g}")
                nc.vector.reciprocal(rec[:], ou_ps[:, :, :, H:H + 1])
                ot = op.tile([128, 2, 2, H], F32, tag="ot", name=f"ot{g}")
                _bmul(nc, ot[:], ou_ps[:, :, :, 0:H], rec[:])
                nc.sync.dma_start(
                    out_h.ap()[2 * g:2 * g + 2].rearrange(
                        "j (u p) h -> p j u h", p=128
                    ),
                    ot[:],
                )
    nc.compile()
    return nc


_NC_CACHE = None


def _get_nc():
    global _NC_CACHE
    if _NC_CACHE is None:
        _NC_CACHE = build_nc()
    return _NC_CACHE


def kernel(inp, Wv, bv, Wk, bk, Wq, bq):
    inp = np.asarray(inp, dtype=np.float32)
    Wv = np.asarray(Wv, dtype=np.float32)
    Wk = np.asarray(Wk, dtype=np.float32)
    Wq = np.asarray(Wq, dtype=np.float32)
    bv = np.asarray(bv, dtype=np.float32)
    bk = np.asarray(bk, dtype=np.float32)
    bq = np.asarray(bq, dtype=np.float32)

    bf16 = ml_dtypes.bfloat16
    wq_b = np.ascontiguousarray(Wq.astype(bf16))
    wk_b = np.ascontiguousarray(Wk.astype(bf16))
    wv_b = np.ascontiguousarray(Wv.astype(bf16))
    bq_c = np.ascontiguousarray(bq.reshape(H, 1))
    bk_c = np.ascontiguousarray(bk.reshape(H, 1))
    bvb = np.ascontiguousarray(np.tile(bv.reshape(1, H), (128, 1)))

    in_maps = []
    for c in range(N_CORES):
        shard = inp[c * NB:(c + 1) * NB]                  # [NB, T, C]
        x_t = np.ascontiguousarray(
            shard.transpose(0, 2, 1).astype(bf16)          # [NB, C, T]
        )
        in_maps.append({
            "x": x_t, "wq": wq_b, "wk": wk_b, "wv": wv_b,
            "bq": bq_c, "bk": bk_c, "bvb": bvb,
        })

    nc = _get_nc()
    res = run_bass_kernel_spmd(nc, in_maps, core_ids=list(range(N_CORES)))
    return np.concatenate(
        [res.results[c]["out"] for c in range(N_CORES)], axis=0
    )
